# revision 32
# baseline (speedup 1.0000x reference)
"""Distributed Trainium2 Bass kernel for causal multi-head attention with RoPE.

Problem: B=2, T=2048, C=1024, H=16 heads, D=64. 8 NeuronCores.

Sharding (2x4 grid): core c handles batch b = c//4 and the 4 heads
g = c%4 -> heads [4g..4g+4). QKV projections + RoPE + causal attention run
fully locally per core in a "transposed" layout (qT/kT = [D_heads, T]) so
no on-chip transposes are ever needed:

  qT = Wq_slice.T @ x.T            (lhsT = Wq natural, rhs = x.T)
  scoresT[k,q] = kT.T-block @ qT   (softmax along PARTITION axis)
  outT = [v|1].T @ exp(scoresT)    (ones column yields softmax denominators)

v4: NO collectives.  The output projection is computed as per-core
PARTIALS: outp[m, q] = Wo_rows(own 256 ch).T @ attn_local for ALL 1024
output channels; the host unshard sums the 4 partials of each batch.
This removes every AllGather (9-38us each, high variance), the gathered
attention reloads, and all cross-queue blocking hazards; the kernel tail
is just  last-norm -> 16 local matmuls -> stores.

The QKV+RoPE projection work (phase A), the attention inner loop
(phase B) and the partial-Wo projection (phase C) are software-pipelined
into one interleaved emission stream; B phases are exp(ACT)-bound, so
phase C's PE work rides in their slack.  Normalization uses a gpsimd
partition_broadcast of the reciprocal denominators.
"""

import numpy as np
import ml_dtypes

import concourse.bacc as bacc
import concourse.mybir as mybir
import concourse.tile as tile
from concourse.bass_utils import run_bass_kernel_spmd

B, T, C, H, D = 2, 2048, 1024, 16, 64
NCORES = 8
HPC = 4              # heads per core
CPC = HPC * D        # channels per core (256)
NPAIR = 2            # head pairs per core
QC = 4               # q-chunks of 512
KB = T // 128        # k-blocks of 128
CCH = C // 128       # contraction chunks of 128
F32 = mybir.dt.float32
BF16 = mybir.dt.bfloat16
AF = mybir.ActivationFunctionType

_cache = {}


def _build_nc():
    nc = bacc.Bacc(None, target_bir_lowering=False, debug=False, num_devices=NCORES)

    # weights and xT are pre-rearranged on the HOST into the exact SBUF
    # layout ([partition, chunk-major free dim]) so every DMA is a plain
    # contiguous slice: bigger descriptors, cheaper triggers
    xT = nc.declare_dram_parameter("xT", [128, 4 * CCH * 512], BF16, isOutput=False)
    wq = nc.declare_dram_parameter("wq", [128, CCH * CPC], BF16, isOutput=False)
    wk = nc.declare_dram_parameter("wk", [128, CCH * CPC], BF16, isOutput=False)
    wv = nc.declare_dram_parameter("wv", [128, CCH * CPC], BF16, isOutput=False)
    # Wo ROW slice (this core's 256 attn channels x all 1024 out channels)
    # for the collective-free partial output projection
    wo = nc.declare_dram_parameter("wo", [128, NPAIR * C], BF16, isOutput=False)
    cosP = nc.declare_dram_parameter("cosP", [128, T], BF16, isOutput=False)
    sinP = nc.declare_dram_parameter("sinP", [128, T], BF16, isOutput=False)
    maskut = nc.declare_dram_parameter("maskut", [128, 256], BF16, isOutput=False)
    smat = nc.declare_dram_parameter("smat", [128, 128], BF16, isOutput=False)
    outp = [nc.declare_dram_parameter(f"outp{qc}", [C, 512], BF16, isOutput=True)
            for qc in range(QC)]

    with tile.TileContext(nc) as tc:
        with (
            tc.tile_pool(name="resident", bufs=1) as rp,
            tc.tile_pool(name="rope", bufs=2) as ropep,
            tc.tile_pool(name="expp", bufs=8) as expp,
            tc.tile_pool(name="outb", bufs=6) as outbp,
            tc.tile_pool(name="normp", bufs=2) as normp,
            tc.tile_pool(name="osbp", bufs=4) as osbp,
            tc.tile_pool(name="psS", bufs=2, space="PSUM") as psS,
            tc.tile_pool(name="psAV", bufs=1, space="PSUM") as psAV,
            tc.tile_pool(name="psA", bufs=2, space="PSUM") as psA,
        ):
            # ---------------- resident SBUF ----------------
            xn = [rp.tile([128, CCH * 512], BF16, name=f"xn{i}") for i in range(4)]
            wqbf = rp.tile([128, CCH * CPC], BF16, name="wqbf")
            wkbf = rp.tile([128, CCH * CPC], BF16, name="wkbf")
            wvbf = rp.tile([128, CCH * CPC], BF16, name="wvbf")
            cos_n = [rp.tile([128, 512], BF16, name=f"cosn{i}") for i in range(4)]
            sin_n = [rp.tile([128, 512], BF16, name=f"sinn{i}") for i in range(4)]
            mask_bf = rp.tile([128, 256], BF16, name="maskbf")
            smat_bf = rp.tile([128, 128], BF16, name="smatbf")
            qTn = [rp.tile([128, NPAIR * 512], BF16, name=f"qTn{i}") for i in range(4)]
            kTn = [rp.tile([128, NPAIR * 512], BF16, name=f"kTn{i}") for i in range(4)]
            # per nch: [v | 1] per head per local k-block (4 blocks of 128)
            vsbn = [rp.tile([128, HPC * 4 * 65], BF16, name=f"vsbn{i}") for i in range(4)]
            wobf = rp.tile([128, NPAIR * C], BF16, name="wobf")

            # ---------------- initial loads, priority-ordered ----------------
            def wload(eng, sb, w, c0, c1):
                eng.dma_start(sb[:, c0 * CPC:c1 * CPC], w[:, c0 * CPC:c1 * CPC])

            def xload(eng, nch, c0, c1):
                eng.dma_start(
                    xn[nch][:, c0 * 512:c1 * 512],
                    xT[:, (nch * CCH + c0) * 512:(nch * CCH + c1) * 512])

            # critical path: wq, x(nch0), cos0/sin0, smat -> compute ASAP.
            # 1-chunk batches spread over the three trigger engines land on
            # many physical queues in parallel
            nc.scalar.dma_start(smat_bf[:], smat[:])
            for c in range(8):
                # chunk-interleaved in consumption order: the cc=0 matmul can
                # start as soon as the first two 1-chunk transfers land
                wload((nc.sync, nc.scalar, nc.gpsimd)[c % 3], wqbf, wq, c, c + 1)
                xload((nc.scalar, nc.gpsimd, nc.sync)[c % 3], 0, c, c + 1)
            nc.scalar.dma_start(cos_n[0][:], cosP[:, 0:512])
            nc.sync.dma_start(sin_n[0][:], sinP[:, 0:512])
            for i in range(4):
                nc.gpsimd.memset(vsbn[i][:], 1.0)

            # stage 2: remaining loads in rough need-order
            wload(nc.sync, wkbf, wk, 0, 2)
            wload(nc.gpsimd, wkbf, wk, 2, 4)
            wload(nc.sync, wkbf, wk, 4, 6)
            wload(nc.gpsimd, wkbf, wk, 6, 8)
            wload(nc.sync, wvbf, wv, 0, 4)
            wload(nc.gpsimd, wvbf, wv, 4, 8)
            xload(nc.sync, 1, 0, 4)
            xload(nc.gpsimd, 1, 4, 8)
            # stage-2 continues on sync/gpsimd only: the scalar queue must
            # stay clear for the phase-A qub copies
            nc.sync.dma_start(cos_n[1][:], cosP[:, 512:1024])
            nc.gpsimd.dma_start(sin_n[1][:], sinP[:, 512:1024])
            nc.sync.dma_start(mask_bf[:], maskut[:])
            xload(nc.sync, 2, 0, 4)
            xload(nc.gpsimd, 2, 4, 8)
            nc.sync.dma_start(cos_n[2][:], cosP[:, 1024:1536])
            nc.gpsimd.dma_start(sin_n[2][:], sinP[:, 1024:1536])
            xload(nc.sync, 3, 0, 4)
            xload(nc.gpsimd, 3, 4, 8)
            nc.sync.dma_start(cos_n[3][:], cosP[:, 1536:2048])
            nc.gpsimd.dma_start(sin_n[3][:], sinP[:, 1536:2048])
            nc.sync.dma_start(wobf[:, 0:C], wo[:, 0:C])
            nc.gpsimd.dma_start(wobf[:, C:2 * C], wo[:, C:2 * C])

            # ---------------- phase A emitters (QKV + RoPE per nch) ----------
            def emit_qk_proj(nch, w_sb, p, tagn):
                ps_t = psA.tile([128, 512], F32, tag="a", name=f"pst{tagn}")
                for cc in range(CCH):
                    nc.tensor.matmul(
                        ps_t[:],
                        w_sb[:, cc * CPC + p * 128: cc * CPC + (p + 1) * 128],
                        xn[nch][:, cc * 512:(cc + 1) * 512],
                        start=(cc == 0), stop=(cc == CCH - 1),
                    )
                return ps_t

            def emit_qk_rope(nch, t_sb, p, ps_t, tagn, rot_pool, rot_tag):
                qub = ropep.tile([128, 512], BF16, tag="qub")
                nc.scalar.copy(qub[:], ps_t[:])
                rot = rot_pool.tile([128, 512], F32, tag=rot_tag, name=f"rot{tagn}")
                nc.tensor.matmul(rot[:], smat_bf[:], qub[:], start=True, stop=True)
                t1 = ropep.tile([128, 512], F32, tag="t1")
                nc.vector.tensor_mul(t1[:], ps_t[:], cos_n[nch][:])
                t2 = ropep.tile([128, 512], F32, tag="t2")
                nc.vector.tensor_mul(t2[:], rot[:], sin_n[nch][:])
                nc.vector.tensor_add(t_sb[:, p * 512:(p + 1) * 512], t1[:], t2[:])

            def emit_qk_group(nch, w_sb, t_sb, p):
                tagn = f"{nch}_{id(w_sb) % 7}_{p}"
                ps_t = emit_qk_proj(nch, w_sb, p, tagn)
                emit_qk_rope(nch, t_sb, p, ps_t, tagn, psA, "a")

            def emit_v_group(nch, tl):
                ps_v = psA.tile([128, 512], F32, tag="a", name=f"psv{nch}_{tl}")
                for cc in range(CCH):
                    nc.tensor.matmul(
                        ps_v[:, 0:CPC],
                        xn[nch][:, cc * 512 + tl * 128: cc * 512 + tl * 128 + 128],
                        wvbf[:, cc * CPC:(cc + 1) * CPC],
                        start=(cc == 0), stop=(cc == CCH - 1),
                    )
                # one strided copy scatters all 4 heads: [128, h, 64]
                dstv = vsbn[nch][:].rearrange("p (h b c) -> p h b c", h=HPC, b=4)[:, :, tl, 0:64]
                srcv = ps_v[:, 0:CPC].rearrange("p (h c) -> p h c", h=HPC)
                nc.vector.tensor_copy(dstv, srcv)

            def gen_A(nch):
                for p in range(NPAIR):
                    emit_qk_group(nch, wqbf, qTn[nch], p)
                    yield
                for p in range(NPAIR):
                    emit_qk_group(nch, wkbf, kTn[nch], p)
                    yield
                for tl in range(4):
                    emit_v_group(nch, tl)
                    yield

            def gen_A0():
                # S0 runs phase A alone, so the rot matmul's wait on the qub
                # copy (ACT round-trip) would bubble the PE between groups.
                # Software-pipeline: group g+1's projection matmuls are
                # emitted before group g's rope.  The extra live PSUM tile
                # (rot) borrows the idle psS pool.  Groups are ordered
                # q0,k0,q1,k1 so B(0) pair-0 can start after two groups.
                specs = [(wqbf, qTn[0], 0), (wkbf, kTn[0], 0),
                         (wqbf, qTn[0], 1), (wkbf, kTn[0], 1)]
                prev = None
                for i, (w_sb, t_sb, p) in enumerate(specs):
                    ps_t = emit_qk_proj(0, w_sb, p, f"a0_{i}")
                    if prev is not None:
                        emit_qk_rope(0, prev[1], prev[2], prev[3], f"a0r{i}",
                                     psS, "s")
                    prev = (w_sb, t_sb, p, ps_t)
                    yield
                emit_qk_rope(0, prev[1], prev[2], prev[3], "a0r_last", psS, "s")
                yield
                for tl in range(4):
                    emit_v_group(0, tl)
                    yield

            # ---------------- phase B emitters (attention) --------------------
            def emit_scores(qc, p, kb, es):
                nqs = max(qc * 512, kb * 128)
                noff = nqs - qc * 512
                n = 512 - noff
                nch = kb // 4
                kl = kb % 4
                ps_s = psS.tile([128, 1024], F32, tag="s", name=f"pss{qc}_{p}_{kb}")
                for i in range(2):
                    hs = slice(i * 64, (i + 1) * 64)
                    nc.tensor.matmul(
                        ps_s[:, i * 512: i * 512 + n],
                        kTn[nch][hs, p * 512 + kl * 128: p * 512 + kl * 128 + 128],
                        qTn[qc][hs, p * 512 + noff: p * 512 + 512],
                        start=True, stop=True,
                        tile_position=(i * 64, 0),
                    )
                e = expp.tile([128, 1024], BF16, tag="e", name=f"e{qc}_{p}_{kb}")
                if noff:
                    # one strided ACTIVATE covers both heads' valid regions
                    nc.scalar.activation(
                        e[:].rearrange("p (b c) -> p b c", b=2)[:, :, 0:n],
                        ps_s[:].rearrange("p (b c) -> p b c", b=2)[:, :, 0:n],
                        AF.Exp, scale=0.125)
                else:
                    nc.scalar.activation(e[:], ps_s[:], AF.Exp, scale=0.125)
                if nqs == kb * 128:  # diagonal block: causal mask
                    ev = e[:].rearrange("p (b c) -> p b c", b=2)[:, :, 0:128]
                    mv = mask_bf[:].rearrange("p (b c) -> p b c", b=2)
                    nc.vector.tensor_mul(ev, ev, mv)
                es[kb] = e

            def emit_av(qc, p, kb, av, e):
                nqs = max(qc * 512, kb * 128)
                noff = nqs - qc * 512
                n = 512 - noff
                nch = kb // 4
                kl = kb % 4
                kmax = 4 * qc + 4
                for i in range(2):
                    h = 2 * p + i
                    vbase = h * 4 * 65 + kl * 65
                    nc.tensor.matmul(
                        av[:, i * 512 + noff: (i + 1) * 512],
                        vsbn[nch][:, vbase: vbase + 65],
                        e[:, i * 512: i * 512 + n],
                        start=(kb == 0), stop=(kb == kmax - 1),
                    )

            _obs = {}

            def emit_norm(qc, p, av):
                # HW quirks: reciprocal_approx_fast misreads PSUM (and the
                # exact reciprocal is ~5x slower), partition_broadcast
                # mis-writes dst at base-partition 64 — so stage the
                # denominators to SBUF and broadcast full-height
                dcp = normp.tile([1, 1024], F32, tag="dcp", name=f"dcp{qc}_{p}")
                nc.vector.tensor_copy(dcp[:], av[64:65, :])
                rec = normp.tile([1, 1024], F32, tag="rec", name=f"rec{qc}_{p}")
                nc.vector.reciprocal_approx_fast(rec[:], dcp[:])
                # one full-height broadcast: every partition gets the whole
                # 1024-wide reciprocal row; the per-head muls slice columns
                bc = normp.tile([128, 1024], F32, tag="bc", name=f"bc{qc}_{p}")
                nc.gpsimd.partition_broadcast(bc[:], rec[:])
                ob = outbp.tile([128, 512], BF16, tag="ob", name=f"ob{qc}_{p}")
                for i in range(2):
                    nc.vector.tensor_mul(ob[i * 64:(i + 1) * 64, :],
                                         av[0:64, i * 512:(i + 1) * 512],
                                         bc[i * 64:(i + 1) * 64, i * 512:(i + 1) * 512])
                _obs[(qc, p)] = ob

            def gen_B(qc, pairs=range(NPAIR)):
                for p in pairs:
                    kmax = 4 * qc + 4
                    lag = 4 if kmax > 4 else 4
                    av = None
                    es = {}
                    for j in range(kmax + lag):
                        if j < kmax:
                            emit_scores(qc, p, j, es)
                        if j >= lag:
                            kb = j - lag
                            if kb == 0:
                                av = psAV.tile([65, 1024], F32, tag="av",
                                               name=f"av{qc}_{p}")
                            emit_av(qc, p, kb, av, es.pop(kb))
                        yield
                    emit_norm(qc, p, av)
                    yield

            # ---------------- phase C emitters (partial Wo per qc) ------------
            # Each core computes partial[m, q] = Wo_rows(own 256 ch).T @ ob for
            # ALL 1024 output channels from purely LOCAL data; the host sums
            # the 4 partials.  16 matmuls per qc — identical PE cost to the
            # gathered form, but with no collective anywhere.
            def gen_Wo_mm(qc):
                for half in range(2):
                    for mch in range(4 * half, 4 * half + 4):
                        ps_o = psA.tile([128, 512], F32, tag="a",
                                        name=f"pso{qc}_{mch}")
                        for p in range(NPAIR):
                            nc.tensor.matmul(
                                ps_o[:],
                                wobf[:, p * C + mch * 128: p * C + (mch + 1) * 128],
                                _obs[(qc, p)][:],
                                start=(p == 0), stop=(p == NPAIR - 1),
                            )
                        osb = osbp.tile([128, 512], BF16, tag="osb",
                                        name=f"osb{qc}_{mch}")
                        # alternate evacuation engine so PSUM drains in parallel
                        if mch % 2 == 0:
                            nc.scalar.copy(osb[:], ps_o[:])
                        else:
                            nc.vector.tensor_copy(osb[:], ps_o[:])
                        if qc == QC - 1:
                            # tail-critical: split stores across 2 queues each
                            nc.sync.dma_start(
                                outp[qc][mch * 128: mch * 128 + 64, :], osb[0:64, :])
                            nc.sync.dma_start(
                                outp[qc][mch * 128 + 64:(mch + 1) * 128, :], osb[64:128, :])
                        else:
                            nc.sync.dma_start(
                                outp[qc][mch * 128:(mch + 1) * 128, :], osb[:])
                    yield

            # ---------------- interleaved emission schedule -------------------
            def weave(streams):
                # streams: list of (generator, n_steps, offset[, span])
                seq = []
                for idx, st in enumerate(streams):
                    g, n, off = st[0], st[1], st[2]
                    span = st[3] if len(st) > 3 else 1.0
                    for k in range(n):
                        seq.append((off + (k + 0.5) / n * span, idx))
                seq.sort(key=lambda x: x[0])
                for _, idx in seq:
                    next(streams[idx][0], None)

            def a_len(nch):
                return 2 * NPAIR + 4

            def b_len(qc):
                return NPAIR * (4 * qc + 4 + 4 + 1)

            # S0: projections for tokens [0:512] (rope-staggered, ordered
            # q0,k0,q1,k1) + B(0) pair 0 woven in as soon as its q/k exist:
            # the exp stream starts ~8us earlier
            weave([(gen_A0(), 9, 0.0), (gen_B(0, [0]), 9, 0.35, 0.75)])
            # S1: A(1) + B(0) pair 1
            weave([(gen_A(1), a_len(1), 0.0), (gen_B(0, [1]), 9, 0.0)])
            # S2: A(2) + B(1) + Wo(0).  B phases are exp-bound, so the Wo
            # matmuls (purely local now) ride in the PE slack.
            weave([(gen_A(2), a_len(2), 0.0), (gen_B(1), b_len(1), 0.0),
                   (gen_Wo_mm(0), 2, 0.3, 0.4)])
            # S3: A(3) + B(2) + Wo(1)
            weave([(gen_A(3), a_len(3), 0.0), (gen_B(2), b_len(2), 0.0),
                   (gen_Wo_mm(1), 2, 0.3, 0.4)])
            # S4: B(3) + first half of Wo(2); Wo(2)'s second half is held
            # back to fill the PE during the final norm chain (below), which
            # also keeps the PE clock warm (HAM) through the tail
            wo2 = gen_Wo_mm(2)
            weave([(gen_B(3), b_len(3), 0.0), (wo2, 1, 0.3, 0.2)])
            # S5: Wo(3).  The p0 half of the first six m-chunks is emitted
            # ahead of the p1 norm (it only needs ob(3,0)), filling the PE
            # while the final norm chain runs; the extra live accumulators
            # borrow the now-idle psS pool ([128,1024] slots hold 2 chunks).
            # second half of Wo(2) first: local work that needs nothing from
            # B(3) pair 1 — fills the PE while the final norm chain runs.
            # It must precede the ps3 psA allocations (ring slots).
            for _ in wo2:
                pass
            ps3 = []
            for sl in range(2):
                wide = psS.tile([128, 1024], F32, tag="s", name=f"pso3w{sl}")
                ps3 += [wide[:, 0:512], wide[:, 512:1024]]
            ps3 += [psA.tile([128, 512], F32, tag="a", name=f"pso3a{m}")
                    for m in range(2)]
            for mch in range(6):
                nc.tensor.matmul(
                    ps3[mch],
                    wobf[:, 0 * C + mch * 128: 0 * C + (mch + 1) * 128],
                    _obs[(3, 0)][:], start=True, stop=False)
            for mch in range(8):
                if mch < 6:
                    ps_o = ps3[mch]
                    nc.tensor.matmul(
                        ps_o,
                        wobf[:, 1 * C + mch * 128: 1 * C + (mch + 1) * 128],
                        _obs[(3, 1)][:], start=False, stop=True)
                else:
                    ps_o = psA.tile([128, 512], F32, tag="a", name=f"pso3b{mch}")
                    for p in range(NPAIR):
                        nc.tensor.matmul(
                            ps_o,
                            wobf[:, p * C + mch * 128: p * C + (mch + 1) * 128],
                            _obs[(3, p)][:], start=(p == 0), stop=(p == 1))
                osb = osbp.tile([128, 512], BF16, tag="osb", name=f"osb3_{mch}")
                if mch % 2 == 0:
                    nc.scalar.copy(osb[:], ps_o)
                else:
                    nc.vector.tensor_copy(osb[:], ps_o)
                nc.sync.dma_start(outp[3][mch * 128: mch * 128 + 64, :], osb[0:64, :])
                nc.sync.dma_start(outp[3][mch * 128 + 64:(mch + 1) * 128, :],
                                  osb[64:128, :])
    return nc


def _get_nc():
    if "nc" not in _cache:
        nc = _build_nc()
        nc.finalize()
        _cache["nc"] = nc
    return _cache["nc"]


def _host_tables(freqs_cos, freqs_sin):
    cosP = np.empty((128, T), np.float32)
    sinP = np.empty((128, T), np.float32)
    for r in range(128):
        i = (r % 64) // 2
        cosP[r] = freqs_cos[:, i]
        sinP[r] = freqs_sin[:, i]
    maskut = np.tile(np.triu(np.ones((128, 128), np.float32)), (1, 2))
    smat = np.zeros((128, 128), np.float32)
    for i in range(64):
        smat[2 * i + 1, 2 * i] = -1.0   # rot[2i] = -q[2i+1]
        smat[2 * i, 2 * i + 1] = 1.0    # rot[2i+1] = +q[2i]
    return cosP, sinP, maskut, smat


def _install_trace_hooks():
    import sys, types
    try:
        import antenv.axon_hooks  # noqa: F401
        return True
    except ImportError:
        pass
    try:
        from trn_agent_boot.trn_boot import _ntff_profile_via_ctypes
        mod = types.ModuleType("antenv.axon_hooks")
        mod._hook = _ntff_profile_via_ctypes("/opt/axon/libaxon_pjrt.so")
        mod.set_axon_ntff_profile_hook = lambda h: setattr(mod, "_hook", h)
        mod.get_axon_ntff_profile_hook = lambda: mod._hook
        sys.modules["antenv.axon_hooks"] = mod
        import antenv
        antenv.axon_hooks = mod
        import concourse.bass_utils as bu
        bu.upload_artifacts = lambda tmpdir: f"file://{tmpdir}"
        return True
    except Exception:
        return False


def _bf16(a):
    return np.ascontiguousarray(a).astype(ml_dtypes.bfloat16)


def _w_sbuf_layout(w):
    # [K*128, N] -> SBUF-resident layout [128, K*N]: chunk cc, partition p
    # holds w[cc*128+p, :]
    k = w.shape[0] // 128
    n = w.shape[1]
    return np.ascontiguousarray(
        w.reshape(k, 128, n).transpose(1, 0, 2).reshape(128, k * n))


def _x_sbuf_layout(xb):
    # [T, C] -> [128, nch*cc*512]: xn[nch][p, cc*512+j] = x[nch*512+j, cc*128+p]
    return np.ascontiguousarray(
        xb.T.reshape(CCH, 128, 4, 512).transpose(1, 2, 0, 3).reshape(128, -1))


def kernel(x, freqs_cos, freqs_sin, Wq, Wk, Wv, Wo, _trace=False):
    x = np.asarray(x, np.float32)
    freqs_cos = np.asarray(freqs_cos, np.float32)
    freqs_sin = np.asarray(freqs_sin, np.float32)
    Wq, Wk, Wv, Wo = (np.asarray(w, np.float32) for w in (Wq, Wk, Wv, Wo))
    cosP, sinP, maskut, smat = _host_tables(freqs_cos, freqs_sin)
    cosP, sinP = _bf16(cosP), _bf16(sinP)

    in_maps = []
    for c in range(NCORES):
        b, g = c // 4, c % 4
        sl = slice(g * CPC, (g + 1) * CPC)
        in_maps.append({
            "xT": _x_sbuf_layout(_bf16(x[b])),
            "wq": _w_sbuf_layout(_bf16(Wq[:, sl])),
            "wk": _w_sbuf_layout(_bf16(Wk[:, sl])),
            "wv": _w_sbuf_layout(_bf16(Wv[:, sl])),
            "wo": _w_sbuf_layout(_bf16(Wo[sl, :])),
            "cosP": cosP, "sinP": sinP,
            "maskut": _bf16(maskut), "smat": _bf16(smat),
        })

    nc = _get_nc()
    if _trace:
        _trace = _install_trace_hooks()
    res = run_bass_kernel_spmd(nc, in_maps, core_ids=list(range(NCORES)), trace=_trace)
    _cache["last_res"] = res

    out = np.empty((B, T, C), np.float32)
    for b in range(B):
        for qc in range(QC):
            acc = np.zeros((C, 512), np.float32)
            for g in range(4):
                acc += np.asarray(res.results[b * 4 + g][f"outp{qc}"], np.float32)
            out[b][qc * 512:(qc + 1) * 512, :] = acc.T
    return out
